# revision 6
# baseline (speedup 1.0000x reference)
"""Trainium2 Bass kernel for nn_FISLayerParameterSharingV2.

Math: dcumsum along an axis with discount d is multiplication by a lower
triangular matrix L[i,j] = d^(i-j).  With H = W = 128 the whole per-(b,t)
chain is expressible as 128x128 matmuls + elementwise products:

    s3  = Ls Z3 Ls^T          (Ls strict lower triangular)
    s2  = Ls (Z2*s3) Ls^T
    out = L  (Z1*s2) L^T      (L inclusive lower triangular)

Kernel layout strategy (per core; B is sharded 4 per core over 8 cores):

  *  "einsum-F": matmul(lhsT=x[b,:,h,:] (c,w), rhs=alphaT (c,3T)) emits
     Z^T tiles [w, 3T] per (b,h) -- channel contraction AND the t->pixel
     layout pivot in a single PE op.  Outputs are packed 4 h-slices per
     PSUM bank and evacuated by ScalarE into a per-b SBUF buffer
     Bp[w, (k,t,h)] (fp16).
  *  Each stage runs in transposed space [w, h]: the left Ls-multiply is a
     PE matmul (contraction over w, per-t stationary), the right Ls^T
     multiply is a discounted inclusive scan along free h on VectorE
     (tensor_tensor_scan, multiplier tensor with 0 at h=0 to reset per
     t-block), which also performs the PSUM->SBUF evacuation.  The
     strict-shift is an AP offset into the scan result + a column memset;
     the leftover d factors fold into the next stage's stationary matrix.
  *  Elementwise Z*s products run on GpSimd (SBUF only).
  *  Final stage: N1 = scan(M1); matmul(lhsT=N1, rhs=d*L^T) un-transposes
     back to [h, w] for free; ScalarE evacuates with the 2^20 unscale.
  *  fp16 storage everywhere with power-of-2 prescales folded into the
     alphas (the scans amplify ~50x per stage and would overflow fp16).

All discount-dependent values (stationaries, scan masks) are runtime input
tensors, so the compiled program is independent of the input values.
"""

import sys
import numpy as np

for _p in ("/opt/trn_rl_repo",):
    if _p not in sys.path:
        sys.path.insert(0, _p)

B, T, C, H, W = 32, 32, 64, 128, 128
NCORES = 8
BPC = B // NCORES          # batches per core
KA = 3                     # number of alphas
C1, C2, C3 = 2.0 ** -8, 2.0 ** -6, 2.0 ** -6
UNSCALE = 1.0 / (C1 * C2 * C3)

_CACHE = {}


def _build_module():
    import concourse.bass as bass
    import concourse.mybir as mybir
    import concourse.tile as tile
    from concourse import bacc
    from contextlib import ExitStack

    dt = mybir.dt
    f32, f16 = dt.float32, dt.float16

    nc = bacc.Bacc(
        "TRN2", target_bir_lowering=False, debug=False, num_devices=NCORES
    )
    xs = nc.declare_dram_parameter("xs", [BPC, C, H, W], f32, isOutput=False)
    alphaT = nc.declare_dram_parameter("alphaT", [128, KA * T], f16, isOutput=False)
    stat3T = nc.declare_dram_parameter("stat3T", [128, T * 128], f16, isOutput=False)
    stat2T = nc.declare_dram_parameter("stat2T", [128, T * 128], f16, isOutput=False)
    frhs = nc.declare_dram_parameter("frhs", [128, T * 128], f16, isOutput=False)
    dmask = nc.declare_dram_parameter("dmask", [128, T * 128], f32, isOutput=False)
    out = nc.declare_dram_parameter("out", [BPC, T, H, W], f32, isOutput=True)

    HB = 32                    # h-block size for x streaming
    NHB = H // HB              # 4 h-blocks
    NPAIR = BPC // 2           # 2 b-pairs
    NG = T // 4                # 8 t-quad groups
    MULT = mybir.AluOpType.mult
    ADD = mybir.AluOpType.add
    COPY = mybir.ActivationFunctionType.Copy

    with tile.TileContext(nc) as tc, ExitStack() as ctx:
        const_pool = ctx.enter_context(tc.tile_pool(name="const", bufs=1))
        xpool = ctx.enter_context(tc.tile_pool(name="xp", bufs=4))
        bppool = ctx.enter_context(tc.tile_pool(name="bp", bufs=2))
        ypool = ctx.enter_context(tc.tile_pool(name="yp", bufs=6))
        mpool = ctx.enter_context(tc.tile_pool(name="mp", bufs=6))
        n1pool = ctx.enter_context(tc.tile_pool(name="n1", bufs=4))
        stpool = ctx.enter_context(tc.tile_pool(name="st", bufs=3))
        ps_ein = ctx.enter_context(tc.tile_pool(name="pse", bufs=3, space="PSUM"))
        ps_s3 = ctx.enter_context(tc.tile_pool(name="ps3", bufs=2, space="PSUM"))
        ps_s2 = ctx.enter_context(tc.tile_pool(name="ps2", bufs=2, space="PSUM"))
        ps_fm = ctx.enter_context(tc.tile_pool(name="psf", bufs=1, space="PSUM"))

        # constants
        alpha_t = const_pool.tile([128, KA * T], f16, tag="alpha")
        nc.sync.dma_start(alpha_t[:], alphaT[:])
        s3_t = const_pool.tile([128, T * 128], f16, tag="s3m")
        nc.sync.dma_start(s3_t[:], stat3T[:])
        s2_t = const_pool.tile([128, T * 128], f16, tag="s2m")
        nc.sync.dma_start(s2_t[:], stat2T[:])
        fr_t = const_pool.tile([128, T * 128], f16, tag="frm")
        nc.sync.dma_start(fr_t[:], frhs[:])
        dm_t = const_pool.tile([128, T * 128], f32, tag="dmm")
        nc.sync.dma_start(dm_t[:], dmask[:])

        for pair in range(NPAIR):
            # ---------------- einsum-F + pivot ----------------
            bp_tiles = [
                bppool.tile([128, KA * T * 128], f16, tag="bp", name=f"bp{pair}_{i}")
                for i in range(2)
            ]
            for hb in range(NHB):
                xt = xpool.tile([128, HB * W], f16, tag="x")
                src = xs[2 * pair : 2 * pair + 2, :, hb * HB : (hb + 1) * HB, :]
                nc.gpsimd.dma_start(
                    xt[:], src.rearrange("b c h w -> (b c) (h w)")
                )
                # 4-h groups per b, alternating b within the block for
                # row-group overlap on the PE
                for jg in range(HB // 4):
                    pts = [
                        ps_ein.tile([128, 4 * KA * T], f32, tag="pe", name=f"pe{i}")
                        for i in range(2)
                    ]
                    for j in range(4):
                        for par in range(2):
                            nc.tensor.matmul(
                                pts[par][:, j * 96 : (j + 1) * 96],
                                lhsT=xt[
                                    64 * par : 64 * par + 64,
                                    (jg * 4 + j) * W : (jg * 4 + j + 1) * W,
                                ],
                                rhs=alpha_t[64 * par : 64 * par + 64, :],
                                tile_position=(64 * par, 0),
                                skip_group_check=True,
                            )
                    h0 = hb * HB + jg * 4
                    for par in range(2):
                        # [ (j,4,str 96), (k,3,str 32), (t,32,str 1) ] ->
                        # Bp free idx = k*T*128 + t*128 + (h0+j)
                        src_ap = (
                            pts[par][:]
                            .rearrange("p (j k t) -> p j k t", j=4, k=KA)
                        )
                        dst_ap = (
                            bp_tiles[par][:]
                            .rearrange("p (k t h) -> p k t h", k=KA, t=T)[
                                :, :, :, h0 : h0 + 4
                            ]
                            .rearrange("p k t j -> p j k t")
                        )
                        nc.scalar.copy(dst_ap, src_ap)

            # ---------------- stages ----------------
            for par in range(2):
                b = 2 * pair + par
                bp = bp_tiles[par]
                bpv = bp[:].rearrange("p (k t h) -> p k t h", k=KA, t=T)
                stage_out = [None, None]  # staging tiles for t-halves
                for g in range(NG):
                    t0 = 4 * g
                    # stage 3: P3 = Ls_t @ Z3^T ; scan -> y3
                    p3 = ps_s3.tile([128, 512], f32, tag="p3")
                    for tl in range(4):
                        t = t0 + tl
                        nc.tensor.matmul(
                            p3[:, tl * 128 : (tl + 1) * 128],
                            lhsT=s3_t[:, t * 128 : (t + 1) * 128],
                            rhs=bpv[:, 2, t, :],
                            skip_group_check=True,
                        )
                    y3 = ypool.tile([128, 516], f16, tag="y")
                    nc.vector.tensor_tensor_scan(
                        y3[:, 1:513], dm_t[:, t0 * 128 : t0 * 128 + 512], p3[:],
                        initial=0.0, op0=MULT, op1=ADD,
                    )
                    # M2 = Z2^T * shift(y3); kill the h=0 columns
                    m2 = mpool.tile([128, 512], f16, tag="m")
                    nc.gpsimd.tensor_mul(
                        m2[:].rearrange("p (t h) -> p t h", t=4),
                        bpv[:, 1, t0 : t0 + 4, :],
                        y3[:, 0:512].rearrange("p (t h) -> p t h", t=4),
                    )
                    nc.gpsimd.memset(
                        m2[:].rearrange("p (t h) -> p t h", t=4)[:, :, 0:1], 0.0
                    )
                    # stage 2
                    p2 = ps_s2.tile([128, 512], f32, tag="p2")
                    for tl in range(4):
                        t = t0 + tl
                        nc.tensor.matmul(
                            p2[:, tl * 128 : (tl + 1) * 128],
                            lhsT=s2_t[:, t * 128 : (t + 1) * 128],
                            rhs=m2[:, tl * 128 : (tl + 1) * 128],
                            skip_group_check=True,
                        )
                    y2 = ypool.tile([128, 516], f16, tag="y")
                    nc.vector.tensor_tensor_scan(
                        y2[:, 1:513], dm_t[:, t0 * 128 : t0 * 128 + 512], p2[:],
                        initial=0.0, op0=MULT, op1=ADD,
                    )
                    m1 = mpool.tile([128, 512], f16, tag="m")
                    nc.gpsimd.tensor_mul(
                        m1[:].rearrange("p (t h) -> p t h", t=4),
                        bpv[:, 0, t0 : t0 + 4, :],
                        y2[:, 0:512].rearrange("p (t h) -> p t h", t=4),
                    )
                    nc.gpsimd.memset(
                        m1[:].rearrange("p (t h) -> p t h", t=4)[:, :, 0:1], 0.0
                    )
                    # stage 1: N1 = scan(M1); out = N1^T @ (d L^T)
                    n1 = n1pool.tile([128, 512], f16, tag="n1")
                    nc.vector.tensor_tensor_scan(
                        n1[:], dm_t[:, t0 * 128 : t0 * 128 + 512], m1[:],
                        initial=0.0, op0=MULT, op1=ADD,
                    )
                    pf = ps_fm.tile([128, 512], f32, tag="pf")
                    for tl in range(4):
                        t = t0 + tl
                        nc.tensor.matmul(
                            pf[:, tl * 128 : (tl + 1) * 128],
                            lhsT=n1[:, tl * 128 : (tl + 1) * 128],
                            rhs=fr_t[:, t * 128 : (t + 1) * 128],
                            skip_group_check=True,
                        )
                    half = g // 4
                    if g % 4 == 0:
                        stage_out[half] = stpool.tile([128, 16 * 128], f32, tag="st", name="stg")
                    nc.scalar.activation(
                        stage_out[half][:, (g % 4) * 512 : (g % 4 + 1) * 512],
                        pf[:], COPY, scale=UNSCALE,
                    )
                    if g % 4 == 3:
                        dst = out[b, half * 16 : half * 16 + 16, :, :]
                        nc.sync.dma_start(
                            dst.rearrange("t h w -> h t w"),
                            stage_out[half][:].rearrange(
                                "p (t w) -> p t w", t=16
                            ),
                        )
    nc.compile()
    return nc


def _host_prep(alpha_1, alpha_2, alpha_3, discount):
    alphaT = np.concatenate(
        [alpha_1.T * C1, alpha_2.T * C2, alpha_3.T * C3], axis=1
    ).astype(np.float16)
    alphaT_dup = np.concatenate([alphaT, alphaT], axis=0)  # [128, 96]

    idx = np.arange(H)
    E = idx[:, None] - idx[None, :]
    ds = np.asarray(discount, dtype=np.float64).reshape(T)
    stat3T = np.zeros((128, T * 128), np.float16)
    stat2T = np.zeros((128, T * 128), np.float16)
    frhs = np.zeros((128, T * 128), np.float16)
    dmask = np.zeros((128, T * 128), np.float32)
    for t in range(T):
        d = ds[t]
        P = d ** np.maximum(E, 0)
        L = np.where(E >= 0, P, 0.0)
        Ls = np.where(E >= 1, P, 0.0)
        sl = slice(t * 128, (t + 1) * 128)
        stat3T[:, sl] = Ls.T.astype(np.float16)
        stat2T[:, sl] = (d * Ls).T.astype(np.float16)
        frhs[:, sl] = (d * L.T).astype(np.float16)
        dmask[:, sl] = np.float32(d)
        dmask[:, t * 128] = 0.0
    return alphaT_dup, stat3T, stat2T, frhs, dmask


def kernel(x, alpha_1, alpha_2, alpha_3, discount):
    from concourse.bass_utils import run_bass_kernel_spmd

    x = np.ascontiguousarray(np.asarray(x, dtype=np.float32))
    alphaT_dup, stat3T, stat2T, frhs, dmask = _host_prep(
        np.asarray(alpha_1, np.float32),
        np.asarray(alpha_2, np.float32),
        np.asarray(alpha_3, np.float32),
        discount,
    )

    if "nc" not in _CACHE:
        _CACHE["nc"] = _build_module()
    nc = _CACHE["nc"]

    shared = {
        "alphaT": alphaT_dup,
        "stat3T": stat3T,
        "stat2T": stat2T,
        "frhs": frhs,
        "dmask": dmask,
    }
    in_maps = [
        {"xs": x[i * BPC : (i + 1) * BPC], **shared} for i in range(NCORES)
    ]
    res = run_bass_kernel_spmd(nc, in_maps, core_ids=list(range(NCORES)))
    outs = [res.results[i]["out"] for i in range(NCORES)]
    return np.concatenate(outs, axis=0).astype(np.float32)


if __name__ == "__main__":
    import reference as ref

    inputs = {k: np.asarray(v) for k, v in ref.setup_inputs().items()}
    got = kernel(**inputs)
    print("kernel output shape:", got.shape, got.dtype)


# revision 14
# speedup vs baseline: 806.4994x; 806.4994x over previous
"""Trainium2 Bass kernel for nn_FISLayerParameterSharingV2.

Math: dcumsum along an axis with discount d is multiplication by a lower
triangular matrix L[i,j] = d^(i-j).  With H = W = 128 the whole per-(b,t)
chain is expressible as 128x128 matmuls + elementwise products:

    s3  = Ls Z3 Ls^T          (Ls strict lower triangular)
    s2  = Ls (Z2*s3) Ls^T
    out = L  (Z1*s2) L^T      (L inclusive lower triangular)

Kernel layout strategy (per core; B is sharded 4 per core over 8 cores):

  *  "einsum-F": matmul(lhsT=x[b,:,h,:] (c,w), rhs=alphaT (c,3T)) emits
     Z^T tiles [w, 3T] per (b,h) -- channel contraction AND the t->pixel
     layout pivot in a single PE op.  Outputs are packed 4 h-slices per
     PSUM bank and evacuated by ScalarE into a per-b SBUF buffer
     Bp[w, (k,t,h)] (fp16).
  *  Each stage runs in transposed space [w, h]: the left Ls-multiply is a
     PE matmul (contraction over w, per-t stationary), the right Ls^T
     multiply is a discounted inclusive scan along free h on VectorE
     (tensor_tensor_scan, multiplier tensor with 0 at h=0 to reset per
     t-block), which also performs the PSUM->SBUF evacuation.  The
     strict-shift is an AP offset into the scan result + a column memset;
     the leftover d factors fold into the next stage's stationary matrix.
  *  Elementwise Z*s products run on GpSimd (SBUF only).
  *  Final stage: N1 = scan(M1); matmul(lhsT=N1, rhs=d*L^T) un-transposes
     back to [h, w] for free; ScalarE evacuates with the 2^20 unscale.
  *  fp16 storage everywhere with power-of-2 prescales folded into the
     alphas (the scans amplify ~50x per stage and would overflow fp16).

All discount-dependent values (stationaries, scan masks) are runtime input
tensors, so the compiled program is independent of the input values.
"""

import sys
import numpy as np

for _p in ("/opt/trn_rl_repo",):
    if _p not in sys.path:
        sys.path.insert(0, _p)

B, T, C, H, W = 32, 32, 64, 128, 128
NCORES = 8
BPC = B // NCORES          # batches per core
KA = 3                     # number of alphas
C1, C2, C3 = 2.0 ** -8, 2.0 ** -6, 2.0 ** -6
UNSCALE = 1.0 / (C1 * C2 * C3)

_CACHE = {}


def _build_module(uniform_d=False):
    import concourse.bass as bass
    import concourse.mybir as mybir
    import concourse.tile as tile
    from concourse import bacc
    from contextlib import ExitStack

    dt = mybir.dt
    f32, f16 = dt.float32, dt.float16

    nc = bacc.Bacc(
        "TRN2", target_bir_lowering=False, debug=False, num_devices=NCORES
    )
    xs = nc.declare_dram_parameter("xs", [BPC, C, H, W], f32, isOutput=False)
    alphaT = nc.declare_dram_parameter("alphaT", [128, KA * T], f16, isOutput=False)
    stat3T = nc.declare_dram_parameter("stat3T", [128, T * 128], f16, isOutput=False)
    frhs = nc.declare_dram_parameter("frhs", [128, T * 128], f16, isOutput=False)
    dmask = nc.declare_dram_parameter("dmask", [128, T * 128], f32, isOutput=False)
    out = nc.declare_dram_parameter("out", [BPC, T, H, W], f32, isOutput=True)

    HB = 32                    # h-block size for x streaming
    NHB = H // HB              # 4 h-blocks
    NPAIR = BPC // 2           # 2 b-pairs
    NG = T // 4                # 8 t-quad groups
    MULT = mybir.AluOpType.mult
    ADD = mybir.AluOpType.add
    COPY = mybir.ActivationFunctionType.Copy

    with tile.TileContext(nc) as tc, ExitStack() as ctx:
        const_pool = ctx.enter_context(tc.tile_pool(name="const", bufs=1))
        xpool = ctx.enter_context(tc.tile_pool(name="xp", bufs=4))
        bppool = ctx.enter_context(tc.tile_pool(name="bp", bufs=4))
        ypool = ctx.enter_context(tc.tile_pool(name="yp", bufs=6))
        mpool = ctx.enter_context(tc.tile_pool(name="mp", bufs=6))
        n1pool = ctx.enter_context(tc.tile_pool(name="n1", bufs=4))
        stpool = ctx.enter_context(tc.tile_pool(name="st", bufs=3))
        pspool = ctx.enter_context(tc.tile_pool(name="ps", bufs=8, space="PSUM"))

        # constants
        alpha_t = const_pool.tile([128, KA * T], f16, tag="alpha")
        nc.sync.dma_start(alpha_t[:], alphaT[:])
        s3_t = const_pool.tile([128, T * 128], f16, tag="s3m")
        nc.sync.dma_start(s3_t[:], stat3T[:])
        fr_t = const_pool.tile([128, T * 128], f16, tag="frm")
        nc.sync.dma_start(fr_t[:], frhs[:])
        dm_t = const_pool.tile([128, T * 128], f32, tag="dmm")
        nc.sync.dma_start(dm_t[:], dmask[:])

        bp_tiles = {}   # pair -> [tile, tile]
        bpv = {}        # pair -> rearranged views

        def make_bp(pair):
            bp_tiles[pair] = [
                bppool.tile(
                    [128, KA * T * 128], f16, tag="bp", name=f"bp{pair}_{i}"
                )
                for i in range(2)
            ]
            bpv[pair] = [
                t[:].rearrange("p (k t h) -> p k t h", k=KA, t=T)
                for t in bp_tiles[pair]
            ]

        def einsum_units(pair):
            """Generator of closures: x-DMA + (mms, pivot-evac) units."""
            for hb in range(NHB):
                def xdma(pair=pair, hb=hb):
                    xt = xpool.tile([128, HB * W], f16, tag="x", name="xt")
                    src = xs[2 * pair : 2 * pair + 2, :, hb * HB : (hb + 1) * HB, :]
                    nc.gpsimd.dma_start(
                        xt[:], src.rearrange("b c h w -> (b c) (h w)")
                    )
                    return xt
                holder = {}
                def ensure_x(xdma=xdma, holder=holder):
                    if "xt" not in holder:
                        holder["xt"] = xdma()
                    return holder["xt"]
                joff = 0
                for ng in (5, 5, 5, 5, 5, 5, 2):
                    def unit(pair=pair, hb=hb, ng=ng, joff=joff, ensure_x=ensure_x):
                        xt = ensure_x()
                        pts = [
                            pspool.tile(
                                [128, ng * KA * T], f32, tag="ps", name=f"pe{i}"
                            )
                            for i in range(2)
                        ]
                        for j in range(ng):
                            for par in range(2):
                                nc.tensor.matmul(
                                    pts[par][:, j * 96 : (j + 1) * 96],
                                    lhsT=xt[
                                        64 * par : 64 * par + 64,
                                        (joff + j) * W : (joff + j + 1) * W,
                                    ],
                                    rhs=alpha_t[64 * par : 64 * par + 64, :],
                                    tile_position=(64 * par, 0),
                                    skip_group_check=True,
                                )
                        h0 = hb * HB + joff
                        for par in range(2):
                            src_ap = pts[par][:].rearrange(
                                "p (j k t) -> p j k t", j=ng, k=KA
                            )
                            dst_ap = (
                                bp_tiles[pair][par][:]
                                .rearrange("p (k t h) -> p k t h", k=KA, t=T)[
                                    :, :, :, h0 : h0 + ng
                                ]
                                .rearrange("p k t j -> p j k t")
                            )
                            if pair == 0 and par == 0:
                                nc.vector.tensor_copy(dst_ap, src_ap)
                            else:
                                nc.scalar.copy(dst_ap, src_ap)
                    yield unit
                    joff += ng

        # per-(b, group) live state for the staged pipeline
        live = {}

        def s3mms(pair, par, g):
            v = bpv[pair][par]
            t0 = 4 * g
            p3 = pspool.tile([128, 512], f32, tag="ps", name="p3")
            if uniform_d:
                nc.tensor.matmul(
                    p3[:],
                    lhsT=s3_t[:, 0:128],
                    rhs=bp_tiles[pair][par][:][
                        :, (2 * T + t0) * 128 : (2 * T + t0 + 4) * 128
                    ],
                    skip_group_check=True,
                )
            else:
                for tl in range(4):
                    t = t0 + tl
                    nc.tensor.matmul(
                        p3[:, tl * 128 : (tl + 1) * 128],
                        lhsT=s3_t[:, t * 128 : (t + 1) * 128],
                        rhs=v[:, 2, t, :],
                        skip_group_check=True,
                    )
            live[(pair, par, g, "p3")] = p3

        def scan3_mul2(pair, par, g):
            v = bpv[pair][par]
            t0 = 4 * g
            p3 = live.pop((pair, par, g, "p3"))
            y3 = ypool.tile([128, 516], f16, tag="y", name="y3")
            nc.vector.tensor_tensor_scan(
                y3[:, 1:513], dm_t[:, t0 * 128 : t0 * 128 + 512], p3[:],
                initial=0.0, op0=MULT, op1=ADD,
            )
            m2 = mpool.tile([128, 512], f16, tag="m", name="m2")
            nc.gpsimd.tensor_mul(
                m2[:].rearrange("p (t h) -> p t h", t=4),
                v[:, 1, t0 : t0 + 4, :],
                y3[:, 0:512].rearrange("p (t h) -> p t h", t=4),
            )
            nc.gpsimd.memset(
                m2[:].rearrange("p (t h) -> p t h", t=4)[:, :, 0:1], 0.0
            )
            live[(pair, par, g, "m2")] = m2

        def s2mms_scan2_mul1(pair, par, g):
            v = bpv[pair][par]
            t0 = 4 * g
            m2 = live.pop((pair, par, g, "m2"))
            p2 = pspool.tile([128, 512], f32, tag="ps", name="p2")
            if uniform_d:
                nc.tensor.matmul(
                    p2[:], lhsT=s3_t[:, 0:128], rhs=m2[:],
                    skip_group_check=True,
                )
            else:
                for tl in range(4):
                    t = t0 + tl
                    nc.tensor.matmul(
                        p2[:, tl * 128 : (tl + 1) * 128],
                        lhsT=s3_t[:, t * 128 : (t + 1) * 128],
                        rhs=m2[:, tl * 128 : (tl + 1) * 128],
                        skip_group_check=True,
                    )
            y2 = ypool.tile([128, 516], f16, tag="y", name="y2")
            nc.vector.tensor_tensor_scan(
                y2[:, 1:513], dm_t[:, t0 * 128 : t0 * 128 + 512], p2[:],
                initial=0.0, op0=MULT, op1=ADD,
            )
            m1 = mpool.tile([128, 512], f16, tag="m", name="m1")
            nc.vector.tensor_mul(
                m1[:].rearrange("p (t h) -> p t h", t=4),
                v[:, 0, t0 : t0 + 4, :],
                y2[:, 0:512].rearrange("p (t h) -> p t h", t=4),
            )
            nc.vector.memset(
                m1[:].rearrange("p (t h) -> p t h", t=4)[:, :, 0:1], 0.0
            )
            live[(pair, par, g, "m1")] = m1

        def scan1_fmms_evac(pair, par, g):
            b = 2 * pair + par
            t0 = 4 * g
            m1 = live.pop((pair, par, g, "m1"))
            n1 = n1pool.tile([128, 512], f16, tag="n1", name="n1")
            nc.vector.tensor_tensor_scan(
                n1[:], dm_t[:, t0 * 128 : t0 * 128 + 512], m1[:],
                initial=0.0, op0=MULT, op1=ADD,
            )
            pf = pspool.tile([128, 512], f32, tag="ps", name="pf")
            for tl in range(4):
                t = t0 + tl
                nc.tensor.matmul(
                    pf[:, tl * 128 : (tl + 1) * 128],
                    lhsT=n1[:, tl * 128 : (tl + 1) * 128],
                    rhs=fr_t[:, 0:128] if uniform_d
                    else fr_t[:, t * 128 : (t + 1) * 128],
                    skip_group_check=True,
                )
            half = g // 4
            if g % 4 == 0:
                live[(pair, par, half, "st")] = stpool.tile(
                    [128, 16 * 128], f32, tag="st", name="stg"
                )
            stg = live[(pair, par, half, "st")]
            nc.scalar.activation(
                stg[:, (g % 4) * 512 : (g % 4 + 1) * 512],
                pf[:], COPY, scale=UNSCALE,
            )
            if g % 4 == 3:
                del live[(pair, par, half, "st")]
                dst = out[b, half * 16 : half * 16 + 16, :, :]
                nc.sync.dma_start(
                    dst.rearrange("t h w -> h t w"),
                    stg[:].rearrange("p (t w) -> p t w", t=16),
                )

        def stage_ticks(pair, extra=None):
            """Software-pipelined stage emission for one pair; `extra` is an
            iterator of einsum units (next pair) interleaved per tick."""
            for k in range(NG + 3):
                for par in range(2):
                    if k < NG:
                        s3mms(pair, par, k)
                    if 0 <= k - 1 < NG:
                        scan3_mul2(pair, par, k - 1)
                    if 0 <= k - 2 < NG:
                        s2mms_scan2_mul1(pair, par, k - 2)
                    if 0 <= k - 3 < NG:
                        scan1_fmms_evac(pair, par, k - 3)
                if extra is not None:
                    for _ in range(3):
                        u = next(extra, None)
                        if u is not None:
                            u()

        make_bp(0)
        for u in einsum_units(0):
            u()
        make_bp(1)
        it1 = iter(list(einsum_units(1)))
        stage_ticks(0, extra=it1)
        for u in it1:
            u()
        stage_ticks(1)

    nc.compile()
    return nc


def _host_prep(alpha_1, alpha_2, alpha_3, discount):
    ds = np.asarray(discount, dtype=np.float64).reshape(T)
    # stage-2's (d*Ls) and the final-matmul's (d*L^T) d-factors are folded
    # into the alpha_1 columns (everything downstream of them is linear).
    a1scaled = alpha_1.T * (C1 * ds[None, :] ** 2)
    alphaT = np.concatenate(
        [a1scaled, alpha_2.T * C2, alpha_3.T * C3], axis=1
    ).astype(np.float16)
    alphaT_dup = np.concatenate([alphaT, alphaT], axis=0)  # [128, 96]

    idx = np.arange(H)
    E = idx[:, None] - idx[None, :]
    stat3T = np.zeros((128, T * 128), np.float16)
    frhs = np.zeros((128, T * 128), np.float16)
    dmask = np.zeros((128, T * 128), np.float32)
    for t in range(T):
        d = ds[t]
        P = d ** np.maximum(E, 0)
        L = np.where(E >= 0, P, 0.0)
        Ls = np.where(E >= 1, P, 0.0)
        sl = slice(t * 128, (t + 1) * 128)
        stat3T[:, sl] = Ls.T.astype(np.float16)
        frhs[:, sl] = L.T.astype(np.float16)
        dmask[:, sl] = np.float32(d)
        dmask[:, t * 128] = 0.0
    return alphaT_dup, stat3T, frhs, dmask


def kernel(x, alpha_1, alpha_2, alpha_3, discount):
    from concourse.bass_utils import run_bass_kernel_spmd

    x = np.ascontiguousarray(np.asarray(x, dtype=np.float32))
    alphaT_dup, stat3T, frhs, dmask = _host_prep(
        np.asarray(alpha_1, np.float32),
        np.asarray(alpha_2, np.float32),
        np.asarray(alpha_3, np.float32),
        discount,
    )

    # NOTE: a batched-matmul variant for uniform discounts (one 512-wide
    # stage matmul instead of four 128-wide) hit an NRT exec-unit crash on
    # hardware; the per-t path below is proven stable and nearly as fast.
    key = ("nc", False)
    if key not in _CACHE:
        _CACHE[key] = _build_module(uniform_d=False)
    nc = _CACHE[key]

    shared = {
        "alphaT": alphaT_dup,
        "stat3T": stat3T,
        "frhs": frhs,
        "dmask": dmask,
    }
    in_maps = [
        {"xs": x[i * BPC : (i + 1) * BPC], **shared} for i in range(NCORES)
    ]
    res = run_bass_kernel_spmd(nc, in_maps, core_ids=list(range(NCORES)))
    outs = [res.results[i]["out"] for i in range(NCORES)]
    return np.concatenate(outs, axis=0).astype(np.float32)


if __name__ == "__main__":
    import reference as ref

    inputs = {k: np.asarray(v) for k, v in ref.setup_inputs().items()}
    got = kernel(**inputs)
    print("kernel output shape:", got.shape, got.dtype)


# revision 20
# speedup vs baseline: 831.1262x; 1.0305x over previous
"""Trainium2 Bass kernel for nn_FISLayerParameterSharingV2.

Math: dcumsum along an axis with discount d is multiplication by a lower
triangular matrix L[i,j] = d^(i-j).  With H = W = 128 the whole per-(b,t)
chain is expressible as 128x128 matmuls + elementwise products:

    s3  = Ls Z3 Ls^T          (Ls strict lower triangular)
    s2  = Ls (Z2*s3) Ls^T
    out = L  (Z1*s2) L^T      (L inclusive lower triangular)

Kernel layout strategy (per core; B is sharded 4 per core over 8 cores):

  *  "einsum-F": matmul(lhsT=x[b,:,h,:] (c,w), rhs=alphaT (c,3T)) emits
     Z^T tiles [w, 3T] per (b,h) -- channel contraction AND the t->pixel
     layout pivot in a single PE op.  Outputs are packed 4 h-slices per
     PSUM bank and evacuated by ScalarE into a per-b SBUF buffer
     Bp[w, (k,t,h)] (fp16).
  *  Each stage runs in transposed space [w, h]: the left Ls-multiply is a
     PE matmul (contraction over w, per-t stationary), the right Ls^T
     multiply is a discounted inclusive scan along free h on VectorE
     (tensor_tensor_scan, multiplier tensor with 0 at h=0 to reset per
     t-block), which also performs the PSUM->SBUF evacuation.  The
     strict-shift is an AP offset into the scan result + a column memset;
     the leftover d factors fold into the next stage's stationary matrix.
  *  Elementwise Z*s products run on GpSimd (SBUF only).
  *  Final stage: N1 = scan(M1); matmul(lhsT=N1, rhs=d*L^T) un-transposes
     back to [h, w] for free; ScalarE evacuates with the 2^20 unscale.
  *  fp16 storage everywhere with power-of-2 prescales folded into the
     alphas (the scans amplify ~50x per stage and would overflow fp16).

All discount-dependent values (stationaries, scan masks) are runtime input
tensors, so the compiled program is independent of the input values.
"""

import sys
import numpy as np

for _p in ("/opt/trn_rl_repo",):
    if _p not in sys.path:
        sys.path.insert(0, _p)

B, T, C, H, W = 32, 32, 64, 128, 128
NCORES = 8
BPC = B // NCORES          # batches per core
KA = 3                     # number of alphas
C1, C2, C3 = 2.0 ** -8, 2.0 ** -6, 2.0 ** -6
UNSCALE = 1.0 / (C1 * C2 * C3)

_CACHE = {}


def _build_module(uniform_d=False):
    import concourse.bass as bass
    import concourse.mybir as mybir
    import concourse.tile as tile
    from concourse import bacc
    from contextlib import ExitStack

    dt = mybir.dt
    f32, f16 = dt.float32, dt.float16

    nc = bacc.Bacc(
        "TRN2", target_bir_lowering=False, debug=False, num_devices=NCORES
    )
    xs = nc.declare_dram_parameter("xs", [BPC, C, H, W], f32, isOutput=False)
    alphaT = nc.declare_dram_parameter("alphaT", [128, KA * T], f16, isOutput=False)
    stat3T = nc.declare_dram_parameter("stat3T", [128, T * 128], f16, isOutput=False)
    frhs = nc.declare_dram_parameter("frhs", [128, T * 128], f16, isOutput=False)
    dmask = nc.declare_dram_parameter("dmask", [128, T * 128], f32, isOutput=False)
    out = nc.declare_dram_parameter("out", [BPC, T, H, W], f32, isOutput=True)

    HB = 32                    # h-block size for x streaming
    NHB = H // HB              # 4 h-blocks
    NPAIR = BPC // 2           # 2 b-pairs
    NG = T // 4                # 8 t-quad groups
    MULT = mybir.AluOpType.mult
    ADD = mybir.AluOpType.add
    COPY = mybir.ActivationFunctionType.Copy

    with tile.TileContext(nc) as tc, ExitStack() as ctx:
        const_pool = ctx.enter_context(tc.tile_pool(name="const", bufs=1))
        xpool = ctx.enter_context(tc.tile_pool(name="xp", bufs=4))
        bppool = ctx.enter_context(tc.tile_pool(name="bp", bufs=4))
        ypool = ctx.enter_context(tc.tile_pool(name="yp", bufs=6))
        mpool = ctx.enter_context(tc.tile_pool(name="mp", bufs=6))
        n1pool = ctx.enter_context(tc.tile_pool(name="n1", bufs=4))
        stpool = ctx.enter_context(tc.tile_pool(name="st", bufs=3))
        pspool = ctx.enter_context(tc.tile_pool(name="ps", bufs=8, space="PSUM"))

        # constants: alpha first (einsum needs it immediately); the big
        # stage constants are DMA'd after x(b0) so they don't delay it.
        alpha_t = const_pool.tile([128, KA * T], f16, tag="alpha")
        nc.sync.dma_start(alpha_t[:], alphaT[:])
        s3_t = const_pool.tile([128, T * 128], f16, tag="s3m")
        fr_t = const_pool.tile([128, T * 128], f16, tag="frm")
        dm_t = const_pool.tile([128, T * 128], f32, tag="dmm")

        def load_stage_consts():
            # first-group slices first so stage-0 can start ASAP
            nc.sync.dma_start(s3_t[:, 0:512], stat3T[:, 0:512])
            nc.sync.dma_start(dm_t[:, 0:512], dmask[:, 0:512])
            nc.sync.dma_start(fr_t[:, 0:512], frhs[:, 0:512])
            nc.sync.dma_start(s3_t[:, 512:], stat3T[:, 512:])
            nc.sync.dma_start(dm_t[:, 512:], dmask[:, 512:])
            nc.sync.dma_start(fr_t[:, 512:], frhs[:, 512:])

        bp_tiles = {}   # pair -> [tile, tile]
        bpv = {}        # pair -> rearranged views

        def make_bp(pair):
            bp_tiles[pair] = [
                bppool.tile(
                    [128, KA * T * 128], f16, tag="bp", name=f"bp{pair}_{i}"
                )
                for i in range(2)
            ]
            bpv[pair] = [
                t[:].rearrange("p (k t h) -> p k t h", k=KA, t=T)
                for t in bp_tiles[pair]
            ]

        def einsum_units(pair):
            """Generator of closures: x-DMA + (mms, pivot-evac) units."""
            for hb in range(NHB):
                holder = {}

                def ensure_x(pair=pair, hb=hb, holder=holder):
                    if "xt" not in holder:
                        xt = xpool.tile([128, HB * W], f16, tag="x", name="xt")
                        src = xs[
                            2 * pair : 2 * pair + 2, :, hb * HB : (hb + 1) * HB, :
                        ]
                        nc.gpsimd.dma_start(
                            xt[:], src.rearrange("b c h w -> (b c) (h w)")
                        )
                        holder["xt"] = xt
                    return holder["xt"]

                joff = 0
                for ng in (5, 5, 5, 5, 5, 5, 2):
                    def unit(pair=pair, hb=hb, ng=ng, joff=joff, ensure_x=ensure_x):
                        xt = ensure_x()
                        pts = [
                            pspool.tile(
                                [128, ng * KA * T], f32, tag="ps", name=f"pe{i}"
                            )
                            for i in range(2)
                        ]
                        for j in range(ng):
                            for par in range(2):
                                nc.tensor.matmul(
                                    pts[par][:, j * 96 : (j + 1) * 96],
                                    lhsT=xt[
                                        64 * par : 64 * par + 64,
                                        (joff + j) * W : (joff + j + 1) * W,
                                    ],
                                    rhs=alpha_t[64 * par : 64 * par + 64, :],
                                    tile_position=(64 * par, 0),
                                    skip_group_check=True,
                                )
                        h0 = hb * HB + joff
                        for par in range(2):
                            src_ap = pts[par][:].rearrange(
                                "p (j k t) -> p j k t", j=ng, k=KA
                            )
                            dst_ap = (
                                bp_tiles[pair][par][:]
                                .rearrange("p (k t h) -> p k t h", k=KA, t=T)[
                                    :, :, :, h0 : h0 + ng
                                ]
                                .rearrange("p k t j -> p j k t")
                            )
                            if pair == 0 and par == 0:
                                nc.vector.tensor_copy(dst_ap, src_ap)
                            else:
                                nc.scalar.copy(dst_ap, src_ap)
                    yield unit
                    joff += ng

        # per-(pair, par, group) live state for the staged pipeline
        live = {}

        def s3mms(pair, par, g):
            v = bpv[pair][par]
            t0 = 4 * g
            p3 = pspool.tile([128, 512], f32, tag="ps", name="p3")
            for tl in range(4):
                t = t0 + tl
                nc.tensor.matmul(
                    p3[:, tl * 128 : (tl + 1) * 128],
                    lhsT=s3_t[:, t * 128 : (t + 1) * 128],
                    rhs=v[:, 2, t, :],
                    skip_group_check=True,
                )
            live[(pair, par, g, "p3")] = p3

        def scan3_mul2(pair, par, g):
            v = bpv[pair][par]
            t0 = 4 * g
            p3 = live.pop((pair, par, g, "p3"))
            y3 = ypool.tile([128, 516], f16, tag="y", name="y3")
            nc.vector.tensor_tensor_scan(
                y3[:, 1:513], dm_t[:, t0 * 128 : t0 * 128 + 512], p3[:],
                initial=0.0, op0=MULT, op1=ADD,
            )
            m2 = mpool.tile([128, 512], f16, tag="m", name="m2")
            nc.gpsimd.tensor_mul(
                m2[:].rearrange("p (t h) -> p t h", t=4),
                v[:, 1, t0 : t0 + 4, :],
                y3[:, 0:512].rearrange("p (t h) -> p t h", t=4),
            )
            nc.gpsimd.memset(
                m2[:].rearrange("p (t h) -> p t h", t=4)[:, :, 0:1], 0.0
            )
            live[(pair, par, g, "m2")] = m2

        def s2mms_scan2_mul1(pair, par, g):
            v = bpv[pair][par]
            t0 = 4 * g
            m2 = live.pop((pair, par, g, "m2"))
            p2 = pspool.tile([128, 512], f32, tag="ps", name="p2")
            for tl in range(4):
                t = t0 + tl
                nc.tensor.matmul(
                    p2[:, tl * 128 : (tl + 1) * 128],
                    lhsT=s3_t[:, t * 128 : (t + 1) * 128],
                    rhs=m2[:, tl * 128 : (tl + 1) * 128],
                    skip_group_check=True,
                )
            y2 = ypool.tile([128, 516], f16, tag="y", name="y2")
            nc.vector.tensor_tensor_scan(
                y2[:, 1:513], dm_t[:, t0 * 128 : t0 * 128 + 512], p2[:],
                initial=0.0, op0=MULT, op1=ADD,
            )
            m1 = mpool.tile([128, 512], f16, tag="m", name="m1")
            nc.vector.tensor_mul(
                m1[:].rearrange("p (t h) -> p t h", t=4),
                v[:, 0, t0 : t0 + 4, :],
                y2[:, 0:512].rearrange("p (t h) -> p t h", t=4),
            )
            nc.vector.memset(
                m1[:].rearrange("p (t h) -> p t h", t=4)[:, :, 0:1], 0.0
            )
            live[(pair, par, g, "m1")] = m1

        def scan1_fmms_evac(pair, par, g):
            b = 2 * pair + par
            t0 = 4 * g
            m1 = live.pop((pair, par, g, "m1"))
            n1 = n1pool.tile([128, 512], f16, tag="n1", name="n1")
            nc.vector.tensor_tensor_scan(
                n1[:], dm_t[:, t0 * 128 : t0 * 128 + 512], m1[:],
                initial=0.0, op0=MULT, op1=ADD,
            )
            pf = pspool.tile([128, 512], f32, tag="ps", name="pf")
            for tl in range(4):
                t = t0 + tl
                nc.tensor.matmul(
                    pf[:, tl * 128 : (tl + 1) * 128],
                    lhsT=n1[:, tl * 128 : (tl + 1) * 128],
                    rhs=fr_t[:, t * 128 : (t + 1) * 128],
                    skip_group_check=True,
                )
            half = g // 4
            if g % 4 == 0:
                live[(pair, par, half, "st")] = stpool.tile(
                    [128, 16 * 128], f32, tag="st", name="stg"
                )
            stg = live[(pair, par, half, "st")]
            nc.scalar.activation(
                stg[:, (g % 4) * 512 : (g % 4 + 1) * 512],
                pf[:], COPY, scale=UNSCALE,
            )
            if g % 4 == 3:
                del live[(pair, par, half, "st")]
                dst = out[b, half * 16 : half * 16 + 16, :, :]
                nc.sync.dma_start(
                    dst.rearrange("t h w -> h t w"),
                    stg[:].rearrange("p (t w) -> p t w", t=16),
                )

        def stage_ticks(pair, extra=None):
            for k in range(NG + 3):
                for par in range(2):
                    if k < NG:
                        s3mms(pair, par, k)
                    if 0 <= k - 1 < NG:
                        scan3_mul2(pair, par, k - 1)
                    if 0 <= k - 2 < NG:
                        s2mms_scan2_mul1(pair, par, k - 2)
                    if 0 <= k - 3 < NG:
                        scan1_fmms_evac(pair, par, k - 3)
                if extra is not None:
                    for _ in range(3):
                        u = next(extra, None)
                        if u is not None:
                            u()

        make_bp(0)
        units0 = list(einsum_units(0))
        units0[0]()
        load_stage_consts()
        for u in units0[1:]:
            u()
        make_bp(1)
        it1 = iter(list(einsum_units(1)))
        stage_ticks(0, extra=it1)
        for u in it1:
            u()
        stage_ticks(1)

    nc.compile()
    return nc


def _host_prep(alpha_1, alpha_2, alpha_3, discount):
    ds = np.asarray(discount, dtype=np.float64).reshape(T)
    # stage-2's (d*Ls) and the final-matmul's (d*L^T) d-factors are folded
    # into the alpha_1 columns (everything downstream of them is linear).
    a1scaled = alpha_1.T * (C1 * ds[None, :] ** 2)
    alphaT = np.concatenate(
        [a1scaled, alpha_2.T * C2, alpha_3.T * C3], axis=1
    ).astype(np.float16)
    alphaT_dup = np.concatenate([alphaT, alphaT], axis=0)  # [128, 96]

    idx = np.arange(H)
    E = idx[:, None] - idx[None, :]
    stat3T = np.zeros((128, T * 128), np.float16)
    frhs = np.zeros((128, T * 128), np.float16)
    dmask = np.zeros((128, T * 128), np.float32)
    for t in range(T):
        d = ds[t]
        P = d ** np.maximum(E, 0)
        L = np.where(E >= 0, P, 0.0)
        Ls = np.where(E >= 1, P, 0.0)
        sl = slice(t * 128, (t + 1) * 128)
        stat3T[:, sl] = Ls.T.astype(np.float16)
        frhs[:, sl] = L.T.astype(np.float16)
        dmask[:, sl] = np.float32(d)
        dmask[:, t * 128] = 0.0
    return alphaT_dup, stat3T, frhs, dmask


def kernel(x, alpha_1, alpha_2, alpha_3, discount):
    from concourse.bass_utils import run_bass_kernel_spmd

    x = np.ascontiguousarray(np.asarray(x, dtype=np.float32))
    alphaT_dup, stat3T, frhs, dmask = _host_prep(
        np.asarray(alpha_1, np.float32),
        np.asarray(alpha_2, np.float32),
        np.asarray(alpha_3, np.float32),
        discount,
    )

    # NOTE: a batched-matmul variant for uniform discounts (one 512-wide
    # stage matmul instead of four 128-wide) hit an NRT exec-unit crash on
    # hardware; the per-t path below is proven stable and nearly as fast.
    key = ("nc", False)
    if key not in _CACHE:
        _CACHE[key] = _build_module(uniform_d=False)
    nc = _CACHE[key]

    shared = {
        "alphaT": alphaT_dup,
        "stat3T": stat3T,
        "frhs": frhs,
        "dmask": dmask,
    }
    in_maps = [
        {"xs": x[i * BPC : (i + 1) * BPC], **shared} for i in range(NCORES)
    ]
    res = run_bass_kernel_spmd(nc, in_maps, core_ids=list(range(NCORES)))
    outs = [res.results[i]["out"] for i in range(NCORES)]
    return np.concatenate(outs, axis=0).astype(np.float32)


if __name__ == "__main__":
    import reference as ref

    inputs = {k: np.asarray(v) for k, v in ref.setup_inputs().items()}
    got = kernel(**inputs)
    print("kernel output shape:", got.shape, got.dtype)


# revision 24
# speedup vs baseline: 868.7591x; 1.0453x over previous
"""Trainium2 Bass kernel for nn_FISLayerParameterSharingV2.

Math: dcumsum along an axis with discount d is multiplication by a lower
triangular matrix L[i,j] = d^(i-j).  With H = W = 128 the whole per-(b,t)
chain is expressible as 128x128 matmuls + elementwise products:

    s3  = Ls Z3 Ls^T          (Ls strict lower triangular)
    s2  = Ls (Z2*s3) Ls^T
    out = L  (Z1*s2) L^T      (L inclusive lower triangular)

Kernel layout strategy (per core; B is sharded 4 per core over 8 cores):

  *  "einsum-F": matmul(lhsT=x[b,:,h,:] (c,w), rhs=alphaT (c,3T)) emits
     Z^T tiles [w, 3T] per (b,h) -- channel contraction AND the t->pixel
     layout pivot in a single PE op.  Outputs are packed 4 h-slices per
     PSUM bank and evacuated by ScalarE into a per-b SBUF buffer
     Bp[w, (k,t,h)] (fp16).
  *  Each stage runs in transposed space [w, h]: the left Ls-multiply is a
     PE matmul (contraction over w, per-t stationary), the right Ls^T
     multiply is a discounted inclusive scan along free h on VectorE
     (tensor_tensor_scan, multiplier tensor with 0 at h=0 to reset per
     t-block), which also performs the PSUM->SBUF evacuation.  The
     strict-shift is an AP offset into the scan result + a column memset;
     the leftover d factors fold into the next stage's stationary matrix.
  *  Elementwise Z*s products run on GpSimd (SBUF only).
  *  Final stage: N1 = scan(M1); matmul(lhsT=N1, rhs=d*L^T) un-transposes
     back to [h, w] for free; ScalarE evacuates with the 2^20 unscale.
  *  fp16 storage everywhere with power-of-2 prescales folded into the
     alphas (the scans amplify ~50x per stage and would overflow fp16).

All discount-dependent values (stationaries, scan masks) are runtime input
tensors, so the compiled program is independent of the input values.
"""

import sys
import numpy as np

for _p in ("/opt/trn_rl_repo",):
    if _p not in sys.path:
        sys.path.insert(0, _p)

B, T, C, H, W = 32, 32, 64, 128, 128
NCORES = 8
BPC = B // NCORES          # batches per core
KA = 3                     # number of alphas
C1, C2, C3 = 2.0 ** -8, 2.0 ** -6, 2.0 ** -6
UNSCALE = 1.0 / (C1 * C2 * C3)

_CACHE = {}


def _build_module(uniform_d=False):
    import concourse.bass as bass
    import concourse.mybir as mybir
    import concourse.tile as tile
    from concourse import bacc
    from contextlib import ExitStack

    dt = mybir.dt
    f32, f16 = dt.float32, dt.float16

    nc = bacc.Bacc(
        "TRN2", target_bir_lowering=False, debug=False, num_devices=NCORES
    )
    xs = nc.declare_dram_parameter("xs", [BPC, C, H, W], f32, isOutput=False)
    alphaT = nc.declare_dram_parameter("alphaT", [128, KA * T], f16, isOutput=False)
    stat3T = nc.declare_dram_parameter("stat3T", [128, T * 128], f16, isOutput=False)
    frhs = nc.declare_dram_parameter("frhs", [128, T * 128], f16, isOutput=False)
    dmask = nc.declare_dram_parameter("dmask", [128, T * 128], f32, isOutput=False)
    out = nc.declare_dram_parameter("out", [BPC, T, H, W], f32, isOutput=True)

    HB = 32                    # h-block size for x streaming
    NHB = H // HB              # 4 h-blocks
    NPAIR = BPC // 2           # 2 b-pairs
    NG = T // 4                # 8 t-quad groups
    MULT = mybir.AluOpType.mult
    ADD = mybir.AluOpType.add
    COPY = mybir.ActivationFunctionType.Copy

    with tile.TileContext(nc) as tc, ExitStack() as ctx:
        const_pool = ctx.enter_context(tc.tile_pool(name="const", bufs=1))
        xpool = ctx.enter_context(tc.tile_pool(name="xp", bufs=4))
        bppool = ctx.enter_context(tc.tile_pool(name="bp", bufs=4))
        ypool = ctx.enter_context(tc.tile_pool(name="yp", bufs=6))
        mpool = ctx.enter_context(tc.tile_pool(name="mp", bufs=6))
        n1pool = ctx.enter_context(tc.tile_pool(name="n1", bufs=4))
        stpool = ctx.enter_context(tc.tile_pool(name="st", bufs=4))
        pspool = ctx.enter_context(tc.tile_pool(name="ps", bufs=8, space="PSUM"))

        # constants: alpha first (einsum needs it immediately); the big
        # stage constants are DMA'd after x(b0) so they don't delay it.
        alpha_t = const_pool.tile([128, KA * T], f16, tag="alpha")
        nc.sync.dma_start(alpha_t[:], alphaT[:])
        s3_t = const_pool.tile([128, T * 128], f16, tag="s3m")
        fr_t = const_pool.tile([128, T * 128], f16, tag="frm")
        dm_t = const_pool.tile([128, T * 128], f32, tag="dmm")

        def load_stage_consts():
            # first-group slices first so stage-0 can start ASAP
            nc.sync.dma_start(s3_t[:, 0:512], stat3T[:, 0:512])
            nc.sync.dma_start(dm_t[:, 0:512], dmask[:, 0:512])
            nc.sync.dma_start(fr_t[:, 0:512], frhs[:, 0:512])
            nc.sync.dma_start(s3_t[:, 512:], stat3T[:, 512:])
            nc.sync.dma_start(dm_t[:, 512:], dmask[:, 512:])
            nc.sync.dma_start(fr_t[:, 512:], frhs[:, 512:])

        bp_tiles = {}   # pair -> [tile, tile]
        bpv = {}        # pair -> rearranged views

        def make_bp(pair):
            bp_tiles[pair] = [
                bppool.tile(
                    [128, KA * T * 128], f16, tag="bp", name=f"bp{pair}_{i}"
                )
                for i in range(2)
            ]
            bpv[pair] = [
                t[:].rearrange("p (k t h) -> p k t h", k=KA, t=T)
                for t in bp_tiles[pair]
            ]

        def einsum_units(pair):
            """Generator of closures: x-DMA + (mms, pivot-evac) units."""
            for hb in range(NHB):
                holder = {}

                def ensure_x(pair=pair, hb=hb, holder=holder):
                    if "xt" not in holder:
                        xt = xpool.tile([128, HB * W], f16, tag="x", name="xt")
                        src = xs[
                            2 * pair : 2 * pair + 2, :, hb * HB : (hb + 1) * HB, :
                        ]
                        nc.gpsimd.dma_start(
                            xt[:], src.rearrange("b c h w -> (b c) (h w)")
                        )
                        holder["xt"] = xt
                    return holder["xt"]

                joff = 0
                for ui, ng in enumerate((5, 5, 5, 5, 5, 5, 2)):
                    def unit(pair=pair, hb=hb, ng=ng, joff=joff,
                             ensure_x=ensure_x, ui=ui):
                        xt = ensure_x()
                        pts = [
                            pspool.tile(
                                [128, ng * KA * T], f32, tag="ps", name=f"pe{i}"
                            )
                            for i in range(2)
                        ]
                        for j in range(ng):
                            for par in range(2):
                                nc.tensor.matmul(
                                    pts[par][:, j * 96 : (j + 1) * 96],
                                    lhsT=xt[
                                        64 * par : 64 * par + 64,
                                        (joff + j) * W : (joff + j + 1) * W,
                                    ],
                                    rhs=alpha_t[64 * par : 64 * par + 64, :],
                                    tile_position=(64 * par, 0),
                                    skip_group_check=True,
                                )
                        h0 = hb * HB + joff
                        for par in range(2):
                            src_ap = pts[par][:].rearrange(
                                "p (j k t) -> p j k t", j=ng, k=KA
                            )
                            dst_ap = (
                                bp_tiles[pair][par][:]
                                .rearrange("p (k t h) -> p k t h", k=KA, t=T)[
                                    :, :, :, h0 : h0 + ng
                                ]
                                .rearrange("p k t j -> p j k t")
                            )
                            gi = hb * 7 + ui
                            if pair == 0 and par == 0 and gi % 3 != 2:
                                nc.vector.tensor_copy(dst_ap, src_ap)
                            else:
                                nc.scalar.copy(dst_ap, src_ap)
                    yield unit
                    joff += ng

        # per-(pair, par, group) live state for the staged pipeline
        live = {}

        def s3mms(pair, par, g):
            v = bpv[pair][par]
            t0 = 4 * g
            p3 = pspool.tile([128, 512], f32, tag="ps", name="p3")
            for tl in range(4):
                t = t0 + tl
                nc.tensor.matmul(
                    p3[:, tl * 128 : (tl + 1) * 128],
                    lhsT=s3_t[:, t * 128 : (t + 1) * 128],
                    rhs=v[:, 2, t, :],
                    skip_group_check=True,
                )
            live[(pair, par, g, "p3")] = p3

        def scan3_mul2(pair, par, g):
            v = bpv[pair][par]
            t0 = 4 * g
            p3 = live.pop((pair, par, g, "p3"))
            y3 = ypool.tile([128, 516], f16, tag="y", name="y3")
            nc.vector.tensor_tensor_scan(
                y3[:, 1:513], dm_t[:, t0 * 128 : t0 * 128 + 512], p3[:],
                initial=0.0, op0=MULT, op1=ADD,
            )
            m2 = mpool.tile([128, 512], f16, tag="m", name="m2")
            nc.gpsimd.tensor_mul(
                m2[:].rearrange("p (t h) -> p t h", t=4),
                v[:, 1, t0 : t0 + 4, :],
                y3[:, 0:512].rearrange("p (t h) -> p t h", t=4),
            )
            nc.gpsimd.memset(
                m2[:].rearrange("p (t h) -> p t h", t=4)[:, :, 0:1], 0.0
            )
            live[(pair, par, g, "m2")] = m2

        def s2mms_scan2_mul1(pair, par, g):
            v = bpv[pair][par]
            t0 = 4 * g
            m2 = live.pop((pair, par, g, "m2"))
            p2 = pspool.tile([128, 512], f32, tag="ps", name="p2")
            for tl in range(4):
                t = t0 + tl
                nc.tensor.matmul(
                    p2[:, tl * 128 : (tl + 1) * 128],
                    lhsT=s3_t[:, t * 128 : (t + 1) * 128],
                    rhs=m2[:, tl * 128 : (tl + 1) * 128],
                    skip_group_check=True,
                )
            y2 = ypool.tile([128, 516], f16, tag="y", name="y2")
            nc.vector.tensor_tensor_scan(
                y2[:, 1:513], dm_t[:, t0 * 128 : t0 * 128 + 512], p2[:],
                initial=0.0, op0=MULT, op1=ADD,
            )
            m1 = mpool.tile([128, 512], f16, tag="m", name="m1")
            eng = nc.gpsimd if g % 2 == 1 else nc.vector
            eng.tensor_mul(
                m1[:].rearrange("p (t h) -> p t h", t=4),
                v[:, 0, t0 : t0 + 4, :],
                y2[:, 0:512].rearrange("p (t h) -> p t h", t=4),
            )
            eng.memset(
                m1[:].rearrange("p (t h) -> p t h", t=4)[:, :, 0:1], 0.0
            )
            live[(pair, par, g, "m1")] = m1

        def scan1_fmms_evac(pair, par, g):
            b = 2 * pair + par
            t0 = 4 * g
            m1 = live.pop((pair, par, g, "m1"))
            n1 = n1pool.tile([128, 512], f16, tag="n1", name="n1")
            nc.vector.tensor_tensor_scan(
                n1[:], dm_t[:, t0 * 128 : t0 * 128 + 512], m1[:],
                initial=0.0, op0=MULT, op1=ADD,
            )
            pf = pspool.tile([128, 512], f32, tag="ps", name="pf")
            for tl in range(4):
                t = t0 + tl
                nc.tensor.matmul(
                    pf[:, tl * 128 : (tl + 1) * 128],
                    lhsT=n1[:, tl * 128 : (tl + 1) * 128],
                    rhs=fr_t[:, t * 128 : (t + 1) * 128],
                    skip_group_check=True,
                )
            half = g // 2
            if g % 2 == 0:
                live[(pair, par, half, "st")] = stpool.tile(
                    [128, 8 * 128], f32, tag="st", name="stg"
                )
            stg = live[(pair, par, half, "st")]
            nc.scalar.activation(
                stg[:, (g % 2) * 512 : (g % 2 + 1) * 512],
                pf[:], COPY, scale=UNSCALE,
            )
            if g % 2 == 1:
                del live[(pair, par, half, "st")]
                dst = out[b, half * 8 : half * 8 + 8, :, :]
                nc.sync.dma_start(
                    dst.rearrange("t h w -> h t w"),
                    stg[:].rearrange("p (t w) -> p t w", t=8),
                )

        def stage_ticks(pair, extra=None):
            for k in range(NG + 3):
                for par in range(2):
                    if k < NG:
                        s3mms(pair, par, k)
                    if 0 <= k - 1 < NG:
                        scan3_mul2(pair, par, k - 1)
                    if 0 <= k - 2 < NG:
                        s2mms_scan2_mul1(pair, par, k - 2)
                    if 0 <= k - 3 < NG:
                        scan1_fmms_evac(pair, par, k - 3)
                if extra is not None:
                    for _ in range(3):
                        u = next(extra, None)
                        if u is not None:
                            u()

        make_bp(0)
        units0 = list(einsum_units(0))
        units0[0]()
        load_stage_consts()
        for u in units0[1:]:
            u()
        make_bp(1)
        it1 = iter(list(einsum_units(1)))
        stage_ticks(0, extra=it1)
        for u in it1:
            u()
        stage_ticks(1)

    nc.compile()
    return nc


def _host_prep(alpha_1, alpha_2, alpha_3, discount):
    ds = np.asarray(discount, dtype=np.float64).reshape(T)
    # stage-2's (d*Ls) and the final-matmul's (d*L^T) d-factors are folded
    # into the alpha_1 columns (everything downstream of them is linear).
    a1scaled = alpha_1.T * (C1 * ds[None, :] ** 2)
    alphaT = np.concatenate(
        [a1scaled, alpha_2.T * C2, alpha_3.T * C3], axis=1
    ).astype(np.float16)
    alphaT_dup = np.concatenate([alphaT, alphaT], axis=0)  # [128, 96]

    idx = np.arange(H)
    E = idx[:, None] - idx[None, :]
    stat3T = np.zeros((128, T * 128), np.float16)
    frhs = np.zeros((128, T * 128), np.float16)
    dmask = np.zeros((128, T * 128), np.float32)
    for t in range(T):
        d = ds[t]
        P = d ** np.maximum(E, 0)
        L = np.where(E >= 0, P, 0.0)
        Ls = np.where(E >= 1, P, 0.0)
        sl = slice(t * 128, (t + 1) * 128)
        stat3T[:, sl] = Ls.T.astype(np.float16)
        frhs[:, sl] = L.T.astype(np.float16)
        dmask[:, sl] = np.float32(d)
        dmask[:, t * 128] = 0.0
    return alphaT_dup, stat3T, frhs, dmask


def kernel(x, alpha_1, alpha_2, alpha_3, discount):
    from concourse.bass_utils import run_bass_kernel_spmd

    x = np.ascontiguousarray(np.asarray(x, dtype=np.float32))
    alphaT_dup, stat3T, frhs, dmask = _host_prep(
        np.asarray(alpha_1, np.float32),
        np.asarray(alpha_2, np.float32),
        np.asarray(alpha_3, np.float32),
        discount,
    )

    # NOTE: a batched-matmul variant for uniform discounts (one 512-wide
    # stage matmul instead of four 128-wide) hit an NRT exec-unit crash on
    # hardware; the per-t path below is proven stable and nearly as fast.
    key = ("nc", False)
    if key not in _CACHE:
        _CACHE[key] = _build_module(uniform_d=False)
    nc = _CACHE[key]

    shared = {
        "alphaT": alphaT_dup,
        "stat3T": stat3T,
        "frhs": frhs,
        "dmask": dmask,
    }
    in_maps = [
        {"xs": x[i * BPC : (i + 1) * BPC], **shared} for i in range(NCORES)
    ]
    res = run_bass_kernel_spmd(nc, in_maps, core_ids=list(range(NCORES)))
    outs = [res.results[i]["out"] for i in range(NCORES)]
    return np.concatenate(outs, axis=0).astype(np.float32)


if __name__ == "__main__":
    import reference as ref

    inputs = {k: np.asarray(v) for k, v in ref.setup_inputs().items()}
    got = kernel(**inputs)
    print("kernel output shape:", got.shape, got.dtype)
